# revision 4
# baseline (speedup 1.0000x reference)
"""GNN message-passing kernel for 8 Trainium2 NeuronCores.

Math (per reference):
  h   = relu(ef @ W1 + b1)                      [E, H]
  K   = (h @ W2 + b2).reshape(E, G, L)          per-edge [G, L] kernels
  t   = einsum('bnl,ne->bel', x, inc)           gather nodes->edges
  y   = einsum('egl,bel->beg', K, t)            per-edge matvec
  out = relu(einsum('ne,beg->bng', inc, y) + b_gc).reshape(B, N*G)

Distribution: shard the edge dim E across the 8 cores (2000 edges each,
padded to 2048 with zero-incidence edges). Scatter partials are summed
on the host, then bias + relu applied.

v3: the per-edge matvec packs TWO edges per matmul with a block-diagonal
stationary: pairs (e, e+256) within each 512-edge group. kT2 holds
[128=(l,s), 256 j, 128=(g+s*64)] with zeroed off-diagonal blocks; tT2
holds [128=(l,s), b, j]. Matmul out = [(s,g), b] per pair. Transposes
are [128,128] per (j-block, b); Y is staged as [1024 j, (b, s, g)] and
the scatter contracts pairs with a host-permuted incidence (2 matmuls
per j-chunk). Output is staged contiguously; host reassembles.
"""

import numpy as np
import ml_dtypes

import concourse.bass as bass
from concourse import bacc
import concourse.mybir as mybir
import concourse.tile as tile
from concourse.bass_utils import run_bass_kernel_spmd
from concourse.masks import make_identity

B, N, E, L, G, F, H = 64, 500, 16000, 64, 64, 8, 128
NCORES = 8
ELR = E // NCORES       # 2000 real edges per core
EL = 2048               # padded; pad edges have zero incidence columns
EG = 512                # edge group
NGR = EL // EG          # 4 groups
NPAIR = EG // 2         # 256 pairs per group
NJ = EL // 2            # 1024 pair rows in Ystage
EC = 128                # j-chunk (transpose / scatter granularity)
NP = 125                # nodes per n-chunk (500 = 4*125)
NQ = 4                  # n-chunks
BG = B * G              # 4096
F32 = mybir.dt.float32
BF16 = mybir.dt.bfloat16
RELU = mybir.ActivationFunctionType.Relu
IDENT = mybir.ActivationFunctionType.Identity

_CACHE = {}
last_results = None     # BassKernelResults of the most recent run (for test.py)


def _build():
    nc = bacc.Bacc("TRN2", target_bir_lowering=False)
    xT_d = nc.declare_dram_parameter("xT", [N, B * L], BF16, isOutput=False)
    inc_d = nc.declare_dram_parameter("inc", [N, EL], BF16, isOutput=False)
    # incT rows are host-permuted: row s*1024 + jg  ->  edge (jg//256)*512 + s*256 + jg%256
    incT_d = nc.declare_dram_parameter("incT", [EL, N], BF16, isOutput=False)
    efT_d = nc.declare_dram_parameter("efT", [F, EL], BF16, isOutput=False)
    W1_d = nc.declare_dram_parameter("W1", [F, H], BF16, isOutput=False)
    b1_d = nc.declare_dram_parameter("b1", [H, 1], F32, isOutput=False)
    W2_d = nc.declare_dram_parameter("W2", [H, G * L], BF16, isOutput=False)
    b2T_d = nc.declare_dram_parameter("b2T", [H, G * L // H], F32, isOutput=False)
    # out staged raw: [nj, m, np, b8, g]; host reassembles to [B, N, G]
    out_d = nc.declare_dram_parameter("out", [BG // 512, NQ, NP, 8, G], F32,
                                      isOutput=True)
    y_d = nc.dram_tensor("Ystage", [NJ, B * 2 * G], BF16)

    with tile.TileContext(nc) as tc, tc.tile_pool(name="const", bufs=1) as cpool:
        with tc.tile_pool(name="h_ps", bufs=2, space="PSUM") as hps:
            # ---- persistent tiles ----
            xT_sb = cpool.tile([NP, NQ, B * L], BF16)       # 32KB/part
            nc.sync.dma_start(
                out=xT_sb[:, :, :],
                in_=xT_d[:, :].rearrange("(q n) c -> n q c", q=NQ),
            )
            W1_sb = cpool.tile([F, H], BF16)
            nc.sync.dma_start(out=W1_sb[:, :], in_=W1_d[:, :])
            b1_sb = cpool.tile([H, 1], F32)
            nc.sync.dma_start(out=b1_sb[:, :], in_=b1_d[:, :])
            W2_sb = cpool.tile([H, G * L], BF16)            # 8KB/part
            nc.sync.dma_start(out=W2_sb[:, :], in_=W2_d[:, :])
            b2T_sb = cpool.tile([H, G * L // H], F32)
            nc.sync.dma_start(out=b2T_sb[:, :], in_=b2T_d[:, :])
            efT_sb = cpool.tile([F, EL], BF16)
            nc.sync.dma_start(out=efT_sb[:, :], in_=efT_d[:, :])
            hT_sb = cpool.tile([H, EL], BF16)               # 4KB/part
            # block-diag kernel pairs: [(l,s), j, (g + s*64)]; off-diag zero
            kT2 = cpool.tile([2 * L, NPAIR, 2 * G], BF16)   # 64KB/part
            nc.gpsimd.memset(kT2[0:L, :, G:2 * G], 0.0)
            nc.gpsimd.memset(kT2[L:2 * L, :, 0:G], 0.0)
            # gathered pairs: [(l,s), b, j]
            tT2 = cpool.tile([2 * L, B, NPAIR], BF16)       # 32KB/part

            # ---- mlp1: hT = relu(W1.T @ efT + b1), all edges upfront ----
            for c in range(4):
                ph = hps.tile([H, 512], F32)
                nc.tensor.matmul(
                    ph[:, :], lhsT=W1_sb[:, :],
                    rhs=efT_sb[:, c * 512:(c + 1) * 512],
                    start=True, stop=True,
                )
                nc.scalar.activation(
                    hT_sb[:, c * 512:(c + 1) * 512], ph[:, :], RELU,
                    bias=b1_sb[:, 0:1],
                )

        # ---- phase 1 ----
        with (
            tc.tile_pool(name="stream", bufs=1) as spool,
            tc.tile_pool(name="ycp", bufs=1) as ycppool,
            tc.tile_pool(name="yfin", bufs=2) as yfpool,
            tc.tile_pool(name="tid", bufs=1) as idpool,
            tc.tile_pool(name="mlp2_ps", bufs=2, space="PSUM") as mps,
            tc.tile_pool(name="gat_ps", bufs=2, space="PSUM") as gps,
            tc.tile_pool(name="mv_ps", bufs=2, space="PSUM") as vps,
            tc.tile_pool(name="tr_ps", bufs=2, space="PSUM") as tps,
        ):
            ident = idpool.tile([128, 128], BF16)
            make_identity(nc, ident[:, :])
            for gr in range(NGR):
                e0 = gr * EG
                # mlp2 -> kT2 diag blocks (+b2), N=512 pumps
                for mc in range(32):
                    pm = mps.tile([H, EG], F32, tag="m2")
                    nc.tensor.matmul(
                        pm[:, :], lhsT=W2_sb[:, mc * H:(mc + 1) * H],
                        rhs=hT_sb[:, e0:e0 + EG], start=True, stop=True,
                    )
                    # pm rows: [0:64] = g0=2mc (l), [64:128] = g1; cols: e
                    # e-local 0:256 -> s=0 block, 256:512 -> s=1 block
                    for par in (0, 1):
                        g = 2 * mc + par
                        src = pm[par * 64:(par + 1) * 64, :]
                        bias = b2T_sb[par * 64:(par + 1) * 64, mc:mc + 1]
                        d0 = kT2[0:L, :, g]
                        d1 = kT2[L:2 * L, :, G + g]
                        if mc % 2 == 0:
                            nc.scalar.activation(d0, src[:, 0:NPAIR], IDENT,
                                                 bias=bias)
                            nc.vector.tensor_scalar_add(d1, src[:, NPAIR:EG],
                                                        bias)
                        else:
                            nc.vector.tensor_scalar_add(d0, src[:, 0:NPAIR],
                                                        bias)
                            nc.scalar.activation(d1, src[:, NPAIR:EG], IDENT,
                                                 bias=bias)

                # gather -> tT2[(l,s), b, j], N=512 pumps, accum over q
                inc_t = spool.tile([NP, NQ, EG], BF16, tag="inc")
                nc.sync.dma_start(
                    out=inc_t[:, :, :],
                    in_=inc_d[:, e0:e0 + EG].rearrange("(q n) e -> n q e", q=NQ),
                )
                for bp in range(B // 2):
                    pg = gps.tile([2 * L, EG], F32, tag="g")
                    for q in range(NQ):
                        nc.tensor.matmul(
                            pg[:, :],
                            lhsT=xT_sb[:, q, bp * 128:(bp + 1) * 128],
                            rhs=inc_t[:, q, :],
                            start=(q == 0), stop=(q == NQ - 1),
                        )
                    for par in (0, 1):
                        b = 2 * bp + par
                        src = pg[par * 64:(par + 1) * 64, :]
                        d0 = tT2[0:L, b, :]
                        d1 = tT2[L:2 * L, b, :]
                        if (bp + par) % 2 == 0:
                            nc.scalar.copy(d0, src[:, 0:NPAIR])
                            nc.vector.tensor_copy(d1, src[:, NPAIR:EG])
                        else:
                            nc.vector.tensor_copy(d0, src[:, 0:NPAIR])
                            nc.scalar.copy(d1, src[:, NPAIR:EG])

                # paired matvec + [128,128] transposes per 128-j block
                for blk in range(NPAIR // EC):
                    j0 = blk * EC
                    # ycp[(s,g), j, b]
                    ycp = ycppool.tile([2 * G, EC, B], BF16, tag="ycp")
                    for jb in range(EC // 8):
                        pv = vps.tile([2 * G, 8, B], F32, tag="mv")
                        for k in range(8):
                            j = j0 + jb * 8 + k
                            nc.tensor.matmul(
                                pv[:, k, :],
                                lhsT=kT2[:, j, :],
                                rhs=tT2[:, :, j],
                                start=True, stop=True,
                            )
                        if jb % 2 == 0:
                            nc.scalar.copy(
                                ycp[:, jb * 8:(jb + 1) * 8, :], pv[:, :, :])
                        else:
                            nc.vector.tensor_copy(
                                ycp[:, jb * 8:(jb + 1) * 8, :], pv[:, :, :])

                    # transpose [(s,g), j] -> [j, (s,g)] per b; 8 b's share
                    # one psum tile, drained as [128, 1024]
                    yfin = yfpool.tile([EC, B, 2 * G], BF16, tag="yf")
                    for b8 in range(B // 8):
                        pt = tps.tile([EC, 8, 2 * G], BF16, tag="tr")
                        for i in range(8):
                            b = b8 * 8 + i
                            nc.tensor.transpose(
                                pt[:, i, :], ycp[:, :, b], ident[:, :],
                            )
                        if b8 % 2 == 0:
                            nc.vector.tensor_copy(
                                yfin[:, b8 * 8:(b8 + 1) * 8, :], pt[:, :, :])
                        else:
                            nc.scalar.copy(
                                yfin[:, b8 * 8:(b8 + 1) * 8, :], pt[:, :, :])
                    jg0 = gr * NPAIR + j0
                    nc.sync.dma_start(
                        out=y_d[jg0:jg0 + EC, :],
                        in_=yfin[:, :, :],
                    )

        # ---- phase 2: pair-split scatter, PSUM accum over all 16 (jc,s) ----
        with (
            tc.tile_pool(name="p2c", bufs=1) as p2c,
            tc.tile_pool(name="p2rhs", bufs=4) as p2r,
            tc.tile_pool(name="p2o", bufs=3) as p2o,
            tc.tile_pool(name="acc_ps", bufs=8, space="PSUM") as aps,
        ):
            NCJ = NJ // EC      # 8 j-chunks
            incP_sb = p2c.tile([EC, 2, NCJ, N], BF16)       # 16KB/part
            nc.sync.dma_start(
                out=incP_sb[:, :, :, :],
                in_=incT_d[:, :].rearrange("(s c e) n -> e s c n", s=2, c=NCJ),
            )
            rts = {}

            def load(idx):
                njx, jc = divmod(idx, NCJ)
                t = p2r.tile([EC, 8, 2, G], BF16, tag="rhs")
                nc.sync.dma_start(
                    out=t[:, :, :, :],
                    in_=y_d[jc * EC:(jc + 1) * EC,
                            njx * 1024:(njx + 1) * 1024],
                )
                rts[idx] = t

            for i in range(3):
                load(i)
            for nj in range(BG // 512):
                paccs = [aps.tile([NP, 8, G], F32, tag="acc",
                                  name=f"acc{nj}_{m}") for m in range(NQ)]
                for jc in range(NCJ):
                    idx = nj * NCJ + jc
                    if idx + 3 < (BG // 512) * NCJ:
                        load(idx + 3)
                    rt = rts.pop(idx)
                    for s in range(2):
                        for m in range(NQ):
                            nc.tensor.matmul(
                                paccs[m][:, :, :],
                                lhsT=incP_sb[:, s, jc, m * NP:(m + 1) * NP],
                                rhs=rt[:, :, s, :],
                                start=(jc == 0 and s == 0),
                                stop=(jc == NCJ - 1 and s == 1),
                            )
                for m in range(NQ):
                    ot = p2o.tile([NP, 8, G], F32, tag="ostage",
                                  name=f"ost{nj}_{m}")
                    if m % 2 == 0:
                        nc.vector.tensor_copy(ot[:, :, :], paccs[m][:, :, :])
                    else:
                        nc.scalar.copy(ot[:, :, :], paccs[m][:, :, :])
                    nc.sync.dma_start(
                        out=out_d[nj, m, :, :, :],
                        in_=ot[:, :, :],
                    )
    nc.compile()
    return nc


def kernel(x, incidence, ef, W1, b1, W2, b2, b_gc):
    global last_results
    x = np.asarray(x, dtype=np.float32)
    incidence = np.asarray(incidence, dtype=np.float32)
    ef = np.asarray(ef, dtype=np.float32)
    W1 = np.asarray(W1, dtype=np.float32)
    b1 = np.asarray(b1, dtype=np.float32)
    W2 = np.asarray(W2, dtype=np.float32)
    b2 = np.asarray(b2, dtype=np.float32)
    b_gc = np.asarray(b_gc, dtype=np.float32)

    if "nc" not in _CACHE:
        _CACHE["nc"] = _build()
    nc = _CACHE["nc"]

    bf = ml_dtypes.bfloat16
    xT = np.ascontiguousarray(
        x.transpose(1, 0, 2).reshape(N, B * L)).astype(bf)
    inc_bf = incidence.astype(bf)
    incT_f = np.ascontiguousarray(incidence.T)
    efT = np.ascontiguousarray(ef.T).astype(bf)
    b1c = np.ascontiguousarray(b1.reshape(H, 1))
    W2_bf = W2.astype(bf)
    b2T = np.ascontiguousarray(b2.reshape(G * L // H, H).T)

    # pair-permuted incT rows: row s*1024 + jg -> edge (jg//256)*512 + s*256 + jg%256
    jg = np.arange(NJ)
    rows_s0 = (jg // NPAIR) * EG + (jg % NPAIR)
    rows_s1 = rows_s0 + NPAIR

    pad = EL - ELR
    in_maps = []
    for c in range(NCORES):
        es = slice(c * ELR, (c + 1) * ELR)
        incT_pad = np.pad(incT_f[es, :], ((0, pad), (0, 0)))
        incP = np.concatenate([incT_pad[rows_s0], incT_pad[rows_s1]], axis=0)
        in_maps.append({
            "xT": xT,
            "inc": np.ascontiguousarray(
                np.pad(inc_bf[:, es], ((0, 0), (0, pad)))),
            "incT": np.ascontiguousarray(incP).astype(bf),
            "efT": np.ascontiguousarray(
                np.pad(efT[:, es], ((0, 0), (0, pad)))),
            "W1": W1.astype(bf), "b1": b1c, "W2": W2_bf, "b2T": b2T,
        })

    import os
    trace = bool(int(os.environ.get("KERNEL_TRACE", "0")))
    last_results = run_bass_kernel_spmd(
        nc, in_maps, list(range(NCORES)), trace=trace)
    partial = np.zeros((B, N, G), np.float32)
    for r in last_results.results:
        # staged [nj, m, np, b8, g] -> [b, n, g]
        arr = r["out"].reshape(BG // 512, NQ, NP, 8, G)
        partial += arr.transpose(0, 3, 1, 2, 4).reshape(B, N, G)
    out = np.maximum(partial + b_gc.reshape(1, 1, G), 0.0)
    return out.reshape(B, N * G).astype(np.float32)
